# revision 22
# baseline (speedup 1.0000x reference)
"""Trainium2 Bass kernel for nn_Mlp_13099650253522 (BitNet-ternary dense MLP).

  h = gelu(x @ ter_quant(w1).T + b1);  y = h @ ter_quant(w2).T + b2
  ter_quant(w) = clip(round(w / g), -1, 1) * g,  g = mean(|w|) + 1e-5

Strategy (8 NeuronCores, data-parallel over the 64*197=12608 tokens,
1576 tokens/core). Schedule keeps the PE busy end-to-end:

 - All input DMAs are emitted first, in stream order, on the sync
   queue (contiguous slab-major layouts, ~360 GB/s); tile-pool WAR
   deps pace the stream automatically. No explicit DMA chaining.
 - PE warmup: dummy matmuls in one long accumulation group hold the
   HAM clock gate at 2.4 GHz until fc1's first real matmul (~26us).
 - gamma pre-passes read a bf16 copy of each weight matrix (flip
   fraction ~1e-6) as 8 contiguous half-slabs, all SBUF-resident;
   |w| row-sums split across DVE (6 slabs) and ACT (2 slabs, via
   activation-Abs accum_out) so gamma1 lands ~20us. The fp32 weights
   then stream exactly once, quantized on arrival on DVE into
   {-2,0,+2} fp8 (g/2 folded into gelu / output scales).
 - fc1 runs as two half-passes (tokens {0,1}, then {2,3}) per weight
   pair so arrival stays ahead of PE consumption; w2's gamma pass +
   stream + quant all overlap fc1; fc2 runs with everything resident.
 - y tiles drain on two DMA queues (gpsimd/scalar alternating).
"""
import sys

for _p in ("/root/.axon_site", "/root/.axon_site/_ro/trn_rl_repo",
           "/root/.axon_site/_ro/pypackages", "/opt/trn_rl_repo"):
    if _p not in sys.path:
        sys.path.append(_p)

import ml_dtypes
import numpy as np

from concourse import bacc
import concourse.mybir as mybir
from concourse import bass_isa
from concourse.tile import TileContext
from concourse.bass_utils import run_bass_kernel_spmd

FP32 = mybir.dt.float32
BF16 = mybir.dt.bfloat16
FP8 = mybir.dt.float8e4
Act = mybir.ActivationFunctionType
Alu = mybir.AluOpType
AxX = mybir.AxisListType.X

N_CORES = 8
B, S, D, H = 64, 197, 768, 3072
TOK = B * S                 # 12608
TOK_PER = TOK // N_CORES    # 1576
NT = 4                      # token tiles per core
TN = TOK_PER // NT          # 394
KD = D // 128               # 6
KH = H // 128               # 24
NP = KH // 2                # 12 weight block-pairs per matrix
NS = 8                      # gamma half-slabs per matrix
SC = (H // NS) * D // 128   # half-slab free size: 2304
EPS = 1e-5
NDUM = 112                  # PE warmup matmuls


def build():
    nc = bacc.Bacc("TRN2", target_bir_lowering=False, debug=False)
    # bf16 gamma copies, slab-major: [slab, partition, 3*768]
    w1g = nc.declare_dram_parameter("w1g", [NS, 128, SC], BF16, isOutput=False)
    w2g = nc.declare_dram_parameter("w2g", [NS, 128, SC], BF16, isOutput=False)
    # fp32 weights, pair-blocked partition-major:
    # w1p[pr, k, j, kd*128+m] == w1[(2pr+j)*128+m, kd*128+k]
    w1p = nc.declare_dram_parameter("w1p", [NP, 128, 2, D], FP32, isOutput=False)
    # w2p[pr, k, j, dc*128+m] == w2[dc*128+m, (2pr+j)*128+k]
    w2p = nc.declare_dram_parameter("w2p", [NP, 128, 2, D], FP32, isOutput=False)
    # x: xt[t, k, kd, n] == x_core[t*394+n, kd*128+k]
    xt = nc.declare_dram_parameter("xt", [NT, 128, KD, TN], BF16, isOutput=False)
    b1r = nc.declare_dram_parameter("b1r", [128, KH], FP32, isOutput=False)
    b2r = nc.declare_dram_parameter("b2r", [128, KD], FP32, isOutput=False)
    # y out: row (t*6+dc)*128+k, col n == y_core[t*394+n, dc*128+k]
    yt = nc.declare_dram_parameter("yt", [H, TN], BF16, isOutput=True)

    with TileContext(nc) as tc:
        with (
            tc.tile_pool(name="singles", bufs=1) as singles,
            tc.tile_pool(name="gslab", bufs=NS - 1) as gslabp,  # [128,2304] bf16
            tc.tile_pool(name="wring", bufs=4) as wringp,     # [128,2,768] fp32
            tc.tile_pool(name="sgnD", bufs=4) as sgnDp,
            tc.tile_pool(name="t1", bufs=NP) as t1p,          # w1 ternary fp8
            tc.tile_pool(name="t2", bufs=NP) as t2p,          # w2 ternary fp8
            tc.tile_pool(name="xb", bufs=NT) as xbp,          # x bf16, resident
            tc.tile_pool(name="hb", bufs=KH * NT) as hbp,     # gelu out, resident
            tc.tile_pool(name="ysb", bufs=4) as ysbp,
            tc.tile_pool(name="ps1", bufs=5, space="PSUM") as ps1p,
            tc.tile_pool(name="ps2", bufs=3, space="PSUM") as ps2p,
        ):
            # ================= input DMA stream (sync queue, in order) ====
            g1t = []
            for i in range(NS):
                gt = gslabp.tile([128, SC], BF16, tag="gslab")
                nc.sync.dma_start(out=gt, in_=w1g[i, :, :])
                g1t.append(gt)

            w1f = [None] * NP
            wf0 = wringp.tile([128, 2, D], FP32, tag="wf")
            w1f[0] = wf0
            nc.sync.dma_start(out=wf0, in_=w1p[0, :, :, :])

            xb = [None] * NT
            for t in (0, 1):
                xbt = xbp.tile([128, KD, TN], BF16, tag="xb")
                xb[t] = xbt
                nc.sync.dma_start(out=xbt, in_=xt[t, :, :, :])

            for pr in range(1, NP):
                wfh = wringp.tile([128, 2, D], FP32, tag="wf")
                w1f[pr] = wfh
                nc.sync.dma_start(out=wfh, in_=w1p[pr, :, :, :])

            for t in (2, 3):
                xbt = xbp.tile([128, KD, TN], BF16, tag="xb")
                xb[t] = xbt
                nc.sync.dma_start(out=xbt, in_=xt[t, :, :, :])

            g2t = []
            for i in range(NS):
                gt = gslabp.tile([128, SC], BF16, tag="gslab")
                nc.sync.dma_start(out=gt, in_=w2g[i, :, :])
                g2t.append(gt)

            w2f = [None] * NP
            for pr in range(NP):
                wfk = wringp.tile([128, 2, D], FP32, tag="wf")
                w2f[pr] = wfk
                nc.sync.dma_start(out=wfk, in_=w2p[pr, :, :, :])

            # ================= PE warmup ==================================
            dmw = singles.tile([128, TN], BF16, tag="dmw")
            nc.vector.memset(dmw, 0.0)
            wps = ps2p.tile([128, TN], FP32, tag="ps2")
            for i in range(NDUM):
                nc.tensor.matmul(wps, dmw[:, 0:128], dmw,
                                 start=(i == 0), stop=(i == NDUM - 1))

            # ---- gpsimd library pre-warm + biases (gpsimd queue)
            dmy = singles.tile([128, 1], FP32, tag="dmy")
            nc.gpsimd.memset(dmy, 0.0)
            dmy2 = singles.tile([128, 1], FP32, tag="dmy2")
            nc.gpsimd.partition_all_reduce(dmy2, dmy, channels=128,
                                           reduce_op=bass_isa.ReduceOp.add)
            b1sb = singles.tile([128, KH], FP32, tag="b1sb")
            nc.gpsimd.dma_start(out=b1sb, in_=b1r[:, :])
            b2sb = singles.tile([128, KD], FP32, tag="b2sb")
            nc.gpsimd.dma_start(out=b2sb, in_=b2r[:, :])

            # ================= gamma machinery ============================
            ascr = singles.tile([128, SC], BF16, tag="ascr")  # ACT scratch

            def gamma_reduce(slabs, acc_tag, act_set):
                acc = singles.tile([128, NS], FP32, tag=acc_tag)
                for i, gt in enumerate(slabs):
                    if i in act_set:
                        nc.scalar.activation(ascr, gt, Act.Abs,
                                             accum_out=acc[:, i:i + 1])
                    else:
                        nc.vector.tensor_reduce(out=acc[:, i:i + 1], in_=gt,
                                                axis=AxX, op=Alu.add,
                                                apply_absolute_value=True)
                return acc

            def gamma_chain(acc, tag):
                rsum = singles.tile([128, 1], FP32, tag=tag + "_rs")
                nc.vector.tensor_reduce(out=rsum[:, 0:1], in_=acc[:, 0:NS],
                                        axis=AxX, op=Alu.add)
                allr = singles.tile([128, 1], FP32, tag=tag + "_ar")
                nc.gpsimd.partition_all_reduce(allr, rsum, channels=128,
                                               reduce_op=bass_isa.ReduceOp.add)
                gf = singles.tile([128, 1], FP32, tag=tag + "_gf")
                nc.vector.tensor_scalar(
                    out=gf, in0=allr, scalar1=1.0 / (D * H),
                    scalar2=EPS, op0=Alu.mult, op1=Alu.add)
                gh = singles.tile([128, 1], FP32, tag=tag + "_gh")
                nc.vector.tensor_scalar_mul(gh, gf, 0.5)
                gn = singles.tile([128, 1], FP32, tag=tag + "_gn")
                nc.vector.tensor_scalar_mul(gn, gf, -0.5)
                return gf, gh, gn

            def quant_dve(wf, t, gh, gn):
                # t = (w >= g/2) - (w <= -g/2) in {-1, 0, +1}; the FULL
                # gamma is folded into the gelu / output scales downstream
                b = sgnDp.tile(list(wf.shape), FP8, tag="sgnD")
                nc.vector.tensor_scalar(out=b, in0=wf, scalar1=gn[:, 0:1],
                                        scalar2=1.0, op0=Alu.is_le,
                                        op1=Alu.mult)
                nc.vector.scalar_tensor_tensor(out=t, in0=wf,
                                               scalar=gh[:, 0:1], in1=b,
                                               op0=Alu.is_ge,
                                               op1=Alu.subtract)

            # ---- gamma1: DVE slabs {0,1,2,4,6}, ACT slabs {3,5,7}
            acc1 = gamma_reduce(g1t, "acc1", act_set=(3, 5, 6, 7))
            g1f, g1h, g1n = gamma_chain(acc1, "g1")

            # ---- quant (DVE) + fc1 interleaved, lookahead 2 pairs
            t1 = [None] * NP

            def quant1(pr):
                t = t1p.tile([128, 2, D], FP8, tag="t1")
                quant_dve(w1f[pr], t, g1h, g1n)
                t1[pr] = t

            hbt = {}

            def fc1(hc, ts):
                for t in ts:
                    ps = ps1p.tile([128, TN], FP32, tag="ps1")
                    for kd in range(KD):
                        nc.tensor.matmul(
                            ps, t1[hc // 2][:, hc % 2, kd * 128:(kd + 1) * 128],
                            xb[t][:, kd, :],
                            start=(kd == 0), stop=(kd == KD - 1))
                    hbv = hbp.tile([128, TN], BF16, tag="hb")
                    nc.scalar.activation(hbv, ps, Act.Gelu,
                                         bias=b1sb[:, hc:hc + 1],
                                         scale=g1f[:, 0:1])
                    hbt[(hc, t)] = hbv

            LOOK = 2
            for pr in range(LOOK):
                quant1(pr)
            for pr in range(NP):
                if pr + LOOK < NP:
                    quant1(pr + LOOK)
                for hc in (2 * pr, 2 * pr + 1):
                    fc1(hc, (0, 1))

            # ---- w2 gamma (DVE only; ACT is busy with gelu) + quant
            acc2 = gamma_reduce(g2t, "acc2", act_set=())
            g2f, g2h, g2n = gamma_chain(acc2, "g2")

            t2 = [None] * NP

            def quant2(pr):
                t = t2p.tile([128, 2, D], FP8, tag="t2")
                quant_dve(w2f[pr], t, g2h, g2n)
                t2[pr] = t

            for pr in range(LOOK):
                quant2(pr)
            for pr in range(NP):
                if pr + LOOK < NP:
                    quant2(pr + LOOK)
                for hc in (2 * pr, 2 * pr + 1):
                    fc1(hc, (2, 3))

            # ================= fc2 ========================================
            for t in range(NT):
                for dc in range(KD):
                    ps = ps2p.tile([128, TN], FP32, tag="ps2")
                    for kh in range(KH):
                        nc.tensor.matmul(
                            ps, t2[kh // 2][:, kh % 2, dc * 128:(dc + 1) * 128],
                            hbt[(kh, t)],
                            start=(kh == 0), stop=(kh == KH - 1))
                    ysb = ysbp.tile([128, TN], BF16, tag="ysb")
                    nc.vector.tensor_scalar(
                        out=ysb, in0=ps, scalar1=g2f[:, 0:1],
                        scalar2=b2sb[:, dc:dc + 1],
                        op0=Alu.mult, op1=Alu.add)
                    q = nc.gpsimd if (t * KD + dc) % 2 == 0 else nc.scalar
                    q.dma_start(
                        out=yt[(t * KD + dc) * 128:(t * KD + dc + 1) * 128, :],
                        in_=ysb)

    nc.compile()
    return nc


_NC = None


def _get_nc():
    global _NC
    if _NC is None:
        _NC = build()
    return _NC


def kernel(x, w1, b1, w2, b2, _trace=False, _trace_kwargs=None):
    nc = _get_nc()
    x = np.asarray(x, dtype=np.float32)
    w1 = np.asarray(w1, dtype=np.float32)
    b1 = np.asarray(b1, dtype=np.float32)
    w2 = np.asarray(w2, dtype=np.float32)
    b2 = np.asarray(b2, dtype=np.float32)

    # blocked fp32 layouts (see dram param comments)
    w1b = w1.reshape(KH, 128, KD, 128).transpose(0, 3, 2, 1).reshape(H, D)
    w2b = w2.T.reshape(H, D)                            # [3072, 768]
    w1pp = np.ascontiguousarray(
        w1b.reshape(NP, 2, 128, D).transpose(0, 2, 1, 3))
    w2pp = np.ascontiguousarray(
        w2b.reshape(NP, 2, 128, D).transpose(0, 2, 1, 3))
    w1gg = np.ascontiguousarray(
        w1b.reshape(NS, 3, 128, D).transpose(0, 2, 1, 3).reshape(NS, 128, SC)
    ).astype(ml_dtypes.bfloat16)
    w2gg = np.ascontiguousarray(
        w2b.reshape(NS, 3, 128, D).transpose(0, 2, 1, 3).reshape(NS, 128, SC)
    ).astype(ml_dtypes.bfloat16)
    b1r = np.ascontiguousarray(b1.reshape(KH, 128).T)   # [128, 24]
    b2r = np.ascontiguousarray(b2.reshape(KD, 128).T)   # [128, 6]

    x2 = x.reshape(TOK, D)
    in_maps = []
    for c in range(N_CORES):
        xc = x2[c * TOK_PER:(c + 1) * TOK_PER]          # [1576, 768]
        xtc = np.ascontiguousarray(
            xc.reshape(NT, TN, KD, 128).transpose(0, 3, 2, 1)
        ).astype(ml_dtypes.bfloat16)
        in_maps.append({
            "xt": xtc, "w1g": w1gg, "w1p": w1pp, "w2g": w2gg, "w2p": w2pp,
            "b1r": b1r, "b2r": b2r,
        })
    out = run_bass_kernel_spmd(nc, in_maps, list(range(N_CORES)),
                               trace=_trace, **(_trace_kwargs or {}))
    res = out.results
    y = np.empty((TOK, D), dtype=np.float32)
    for c in range(N_CORES):
        ytc = np.asarray(res[c]["yt"]).astype(np.float32)   # [3072, 394]
        y[c * TOK_PER:(c + 1) * TOK_PER] = (
            ytc.reshape(NT, KD, 128, TN).transpose(0, 3, 1, 2).reshape(TOK_PER, D))
    y = y.reshape(B, S, D)
    if _trace:
        return y, out
    return y


# revision 24
# speedup vs baseline: 1.1615x; 1.1615x over previous
"""Trainium2 Bass kernel for nn_Mlp_13099650253522 (BitNet-ternary dense MLP).

  h = gelu(x @ ter_quant(w1).T + b1);  y = h @ ter_quant(w2).T + b2
  ter_quant(w) = clip(round(w / g), -1, 1) * g,  g = mean(|w|) + 1e-5

Strategy (8 NeuronCores, data-parallel over the 64*197=12608 tokens,
1576 tokens/core). Schedule keeps the PE busy end-to-end:

 - All input DMAs are emitted first, in stream order, on the sync
   queue (contiguous slab-major layouts, ~360 GB/s); tile-pool WAR
   deps pace the stream automatically. No explicit DMA chaining.
 - PE warmup: dummy matmuls in one long accumulation group hold the
   HAM clock gate at 2.4 GHz until fc1's first real matmul (~26us).
 - gamma pre-passes read a bf16 copy of each weight matrix (flip
   fraction ~1e-6) as 8 contiguous half-slabs, all SBUF-resident;
   |w| row-sums split across DVE (6 slabs) and ACT (2 slabs, via
   activation-Abs accum_out) so gamma1 lands ~20us. The fp32 weights
   then stream exactly once, quantized on arrival on DVE into
   {-2,0,+2} fp8 (g/2 folded into gelu / output scales).
 - fc1 runs as two half-passes (tokens {0,1}, then {2,3}) per weight
   pair so arrival stays ahead of PE consumption; w2's gamma pass +
   stream + quant all overlap fc1; fc2 runs with everything resident.
 - y tiles drain on two DMA queues (gpsimd/scalar alternating).
"""
import sys

for _p in ("/root/.axon_site", "/root/.axon_site/_ro/trn_rl_repo",
           "/root/.axon_site/_ro/pypackages", "/opt/trn_rl_repo"):
    if _p not in sys.path:
        sys.path.append(_p)

import ml_dtypes
import numpy as np

from concourse import bacc
import concourse.mybir as mybir
from concourse import bass_isa
from concourse.tile import TileContext
from concourse.bass_utils import run_bass_kernel_spmd

FP32 = mybir.dt.float32
BF16 = mybir.dt.bfloat16
FP8 = mybir.dt.float8e4
Act = mybir.ActivationFunctionType
Alu = mybir.AluOpType
AxX = mybir.AxisListType.X

N_CORES = 8
B, S, D, H = 64, 197, 768, 3072
TOK = B * S                 # 12608
TOK_PER = TOK // N_CORES    # 1576
NT = 4                      # token tiles per core
TN = TOK_PER // NT          # 394
KD = D // 128               # 6
KH = H // 128               # 24
NP = KH // 2                # 12 weight block-pairs per matrix
NS = 8                      # gamma half-slabs per matrix
SC = (H // NS) * D // 128   # half-slab free size: 2304
EPS = 1e-5
NDUM = 112                  # PE warmup matmuls


def build():
    nc = bacc.Bacc("TRN2", target_bir_lowering=False, debug=False)
    # bf16 gamma copies, slab-major: [slab, partition, 3*768]
    w1g = nc.declare_dram_parameter("w1g", [NS, 128, SC], BF16, isOutput=False)
    w2g = nc.declare_dram_parameter("w2g", [NS, 128, SC], BF16, isOutput=False)
    # fp32 weights, pair-blocked partition-major:
    # w1p[pr, k, j, kd*128+m] == w1[(2pr+j)*128+m, kd*128+k]
    w1p = nc.declare_dram_parameter("w1p", [NP, 128, 2, D], FP32, isOutput=False)
    # w2p[pr, k, j, dc*128+m] == w2[dc*128+m, (2pr+j)*128+k]
    w2p = nc.declare_dram_parameter("w2p", [NP, 128, 2, D], FP32, isOutput=False)
    # x: xt[t, k, kd, n] == x_core[t*394+n, kd*128+k]
    xt = nc.declare_dram_parameter("xt", [NT, 128, KD, TN], BF16, isOutput=False)
    b1r = nc.declare_dram_parameter("b1r", [128, KH], FP32, isOutput=False)
    b2r = nc.declare_dram_parameter("b2r", [128, KD], FP32, isOutput=False)
    # y out: row (t*6+dc)*128+k, col n == y_core[t*394+n, dc*128+k]
    yt = nc.declare_dram_parameter("yt", [H, TN], BF16, isOutput=True)

    with TileContext(nc) as tc:
        with (
            tc.tile_pool(name="singles", bufs=1) as singles,
            tc.tile_pool(name="gslab", bufs=NS - 1) as gslabp,  # [128,2304] bf16
            tc.tile_pool(name="wring", bufs=4) as wringp,     # [128,2,768] fp32
            tc.tile_pool(name="sgnD", bufs=4) as sgnDp,
            tc.tile_pool(name="t1", bufs=NP) as t1p,          # w1 ternary fp8
            tc.tile_pool(name="t2", bufs=NP) as t2p,          # w2 ternary fp8
            tc.tile_pool(name="xb", bufs=NT) as xbp,          # x bf16, resident
            tc.tile_pool(name="hb", bufs=KH * NT) as hbp,     # gelu out, resident
            tc.tile_pool(name="ysb", bufs=4) as ysbp,
            tc.tile_pool(name="ps1", bufs=5, space="PSUM") as ps1p,
            tc.tile_pool(name="ps2", bufs=3, space="PSUM") as ps2p,
        ):
            # ================= input DMA stream (sync queue, in order) ====
            g1t = []
            for i in range(NS):
                gt = gslabp.tile([128, SC], BF16, tag="gslab")
                nc.sync.dma_start(out=gt, in_=w1g[i, :, :])
                g1t.append(gt)

            w1f = [None] * NP
            wf0 = wringp.tile([128, 2, D], FP32, tag="wf")
            w1f[0] = wf0
            nc.sync.dma_start(out=wf0, in_=w1p[0, :, :, :])

            xb = [None] * NT
            for t in (0, 1):
                xbt = xbp.tile([128, KD, TN], BF16, tag="xb")
                xb[t] = xbt
                nc.sync.dma_start(out=xbt, in_=xt[t, :, :, :])

            for pr in range(1, NP):
                wfh = wringp.tile([128, 2, D], FP32, tag="wf")
                w1f[pr] = wfh
                nc.sync.dma_start(out=wfh, in_=w1p[pr, :, :, :])

            for t in (2, 3):
                xbt = xbp.tile([128, KD, TN], BF16, tag="xb")
                xb[t] = xbt
                nc.sync.dma_start(out=xbt, in_=xt[t, :, :, :])

            g2t = []
            for i in range(NS):
                gt = gslabp.tile([128, SC], BF16, tag="gslab")
                nc.sync.dma_start(out=gt, in_=w2g[i, :, :])
                g2t.append(gt)

            w2f = [None] * NP
            for pr in range(NP):
                wfk = wringp.tile([128, 2, D], FP32, tag="wf")
                w2f[pr] = wfk
                nc.sync.dma_start(out=wfk, in_=w2p[pr, :, :, :])

            # ================= PE warmup ==================================
            dmw = singles.tile([128, TN], BF16, tag="dmw")
            nc.vector.memset(dmw, 0.0)
            wps = ps2p.tile([128, TN], FP32, tag="ps2")
            for i in range(NDUM):
                nc.tensor.matmul(wps, dmw[:, 0:128], dmw,
                                 start=(i == 0), stop=(i == NDUM - 1))

            # ---- gpsimd library pre-warm + biases (gpsimd queue)
            dmy = singles.tile([128, 1], FP32, tag="dmy")
            nc.gpsimd.memset(dmy, 0.0)
            dmy2 = singles.tile([128, 1], FP32, tag="dmy2")
            nc.gpsimd.partition_all_reduce(dmy2, dmy, channels=128,
                                           reduce_op=bass_isa.ReduceOp.add)
            b1sb = singles.tile([128, KH], FP32, tag="b1sb")
            nc.gpsimd.dma_start(out=b1sb, in_=b1r[:, :])
            b2sb = singles.tile([128, KD], FP32, tag="b2sb")
            nc.gpsimd.dma_start(out=b2sb, in_=b2r[:, :])

            # ================= gamma machinery ============================
            ascr = singles.tile([128, SC], BF16, tag="ascr")  # ACT scratch

            def gamma_reduce(slabs, acc_tag, act_set):
                acc = singles.tile([128, NS], FP32, tag=acc_tag)
                for i, gt in enumerate(slabs):
                    if i in act_set:
                        nc.scalar.activation(ascr, gt, Act.Abs,
                                             accum_out=acc[:, i:i + 1])
                    else:
                        nc.vector.tensor_reduce(out=acc[:, i:i + 1], in_=gt,
                                                axis=AxX, op=Alu.add,
                                                apply_absolute_value=True)
                return acc

            def gamma_chain(acc, tag):
                rsum = singles.tile([128, 1], FP32, tag=tag + "_rs")
                nc.vector.tensor_reduce(out=rsum[:, 0:1], in_=acc[:, 0:NS],
                                        axis=AxX, op=Alu.add)
                allr = singles.tile([128, 1], FP32, tag=tag + "_ar")
                nc.gpsimd.partition_all_reduce(allr, rsum, channels=128,
                                               reduce_op=bass_isa.ReduceOp.add)
                gf = singles.tile([128, 1], FP32, tag=tag + "_gf")
                nc.vector.tensor_scalar(
                    out=gf, in0=allr, scalar1=1.0 / (D * H),
                    scalar2=EPS, op0=Alu.mult, op1=Alu.add)
                gh = singles.tile([128, 1], FP32, tag=tag + "_gh")
                nc.vector.tensor_scalar_mul(gh, gf, 0.5)
                gn = singles.tile([128, 1], FP32, tag=tag + "_gn")
                nc.vector.tensor_scalar_mul(gn, gf, -0.5)
                return gf, gh, gn

            def quant_dve(wf, t, gh, gn):
                # t = (w >= g/2) - (w <= -g/2) in {-1, 0, +1}; the FULL
                # gamma is folded into the gelu / output scales downstream
                b = sgnDp.tile(list(wf.shape), FP8, tag="sgnD")
                nc.vector.tensor_scalar(out=b, in0=wf, scalar1=gn[:, 0:1],
                                        scalar2=1.0, op0=Alu.is_le,
                                        op1=Alu.mult)
                nc.vector.scalar_tensor_tensor(out=t, in0=wf,
                                               scalar=gh[:, 0:1], in1=b,
                                               op0=Alu.is_ge,
                                               op1=Alu.subtract)

            # ---- gamma1: DVE slabs {0,1,2,4,6}, ACT slabs {3,5,7}
            acc1 = gamma_reduce(g1t, "acc1", act_set=(3, 5, 6, 7))
            g1f, g1h, g1n = gamma_chain(acc1, "g1")

            # ---- quant (DVE) + fc1 interleaved, lookahead 2 pairs
            t1 = [None] * NP

            def quant1(pr):
                t = t1p.tile([128, 2, D], FP8, tag="t1")
                quant_dve(w1f[pr], t, g1h, g1n)
                t1[pr] = t

            hbt = {}

            def fc1(hc, ts):
                for t in ts:
                    ps = ps1p.tile([128, TN], FP32, tag="ps1")
                    for kd in range(KD):
                        nc.tensor.matmul(
                            ps, t1[hc // 2][:, hc % 2, kd * 128:(kd + 1) * 128],
                            xb[t][:, kd, :],
                            start=(kd == 0), stop=(kd == KD - 1))
                    hbv = hbp.tile([128, TN], BF16, tag="hb")
                    nc.scalar.activation(hbv, ps, Act.Gelu,
                                         bias=b1sb[:, hc:hc + 1],
                                         scale=g1f[:, 0:1])
                    hbt[(hc, t)] = hbv

            LOOK = 2
            for pr in range(LOOK):
                quant1(pr)
            for pr in range(NP):
                if pr + LOOK < NP:
                    quant1(pr + LOOK)
                for hc in (2 * pr, 2 * pr + 1):
                    fc1(hc, (0, 1))

            # ---- w2 gamma (DVE only; ACT is busy with gelu) + quant
            acc2 = gamma_reduce(g2t, "acc2", act_set=())
            g2f, g2h, g2n = gamma_chain(acc2, "g2")

            t2 = [None] * NP

            def quant2(pr):
                t = t2p.tile([128, 2, D], FP8, tag="t2")
                quant_dve(w2f[pr], t, g2h, g2n)
                t2[pr] = t

            for pr in range(LOOK):
                quant2(pr)
            for pr in range(NP):
                if pr + LOOK < NP:
                    quant2(pr + LOOK)
                for hc in (2 * pr, 2 * pr + 1):
                    fc1(hc, (2, 3))

            # ================= fc2 ========================================
            for t in range(NT):
                for dc in range(KD):
                    ps = ps2p.tile([128, TN], FP32, tag="ps2")
                    for kh in range(KH):
                        nc.tensor.matmul(
                            ps, t2[kh // 2][:, kh % 2, dc * 128:(dc + 1) * 128],
                            hbt[(kh, t)],
                            start=(kh == 0), stop=(kh == KH - 1))
                    ysb = ysbp.tile([128, TN], BF16, tag="ysb")
                    nc.vector.tensor_scalar(
                        out=ysb, in0=ps, scalar1=g2f[:, 0:1],
                        scalar2=b2sb[:, dc:dc + 1],
                        op0=Alu.mult, op1=Alu.add)
                    q = nc.gpsimd if (t * KD + dc) % 2 == 0 else nc.scalar
                    q.dma_start(
                        out=yt[(t * KD + dc) * 128:(t * KD + dc + 1) * 128, :],
                        in_=ysb)

    nc.compile()
    return nc


_NC = None


def _get_nc():
    global _NC
    if _NC is None:
        _NC = build()
    return _NC


def kernel(x, w1, b1, w2, b2, _trace=False, _trace_kwargs=None):
    nc = _get_nc()
    x = np.asarray(x, dtype=np.float32)
    w1 = np.asarray(w1, dtype=np.float32)
    b1 = np.asarray(b1, dtype=np.float32)
    w2 = np.asarray(w2, dtype=np.float32)
    b2 = np.asarray(b2, dtype=np.float32)

    # blocked fp32 layouts (see dram param comments)
    w1b = w1.reshape(KH, 128, KD, 128).transpose(0, 3, 2, 1).reshape(H, D)
    w2b = w2.T.reshape(H, D)                            # [3072, 768]
    w1pp = np.ascontiguousarray(
        w1b.reshape(NP, 2, 128, D).transpose(0, 2, 1, 3))
    w2pp = np.ascontiguousarray(
        w2b.reshape(NP, 2, 128, D).transpose(0, 2, 1, 3))
    w1gg = np.ascontiguousarray(
        w1b.reshape(NS, 3, 128, D).transpose(0, 2, 1, 3).reshape(NS, 128, SC)
    ).astype(ml_dtypes.bfloat16)
    w2gg = np.ascontiguousarray(
        w2b.reshape(NS, 3, 128, D).transpose(0, 2, 1, 3).reshape(NS, 128, SC)
    ).astype(ml_dtypes.bfloat16)
    b1r = np.ascontiguousarray(b1.reshape(KH, 128).T)   # [128, 24]
    b2r = np.ascontiguousarray(b2.reshape(KD, 128).T)   # [128, 6]

    x2 = x.reshape(TOK, D)
    in_maps = []
    for c in range(N_CORES):
        xc = x2[c * TOK_PER:(c + 1) * TOK_PER]          # [1576, 768]
        xtc = np.ascontiguousarray(
            xc.reshape(NT, TN, KD, 128).transpose(0, 3, 2, 1)
        ).astype(ml_dtypes.bfloat16)
        in_maps.append({
            "xt": xtc, "w1g": w1gg, "w1p": w1pp, "w2g": w2gg, "w2p": w2pp,
            "b1r": b1r, "b2r": b2r,
        })
    out = run_bass_kernel_spmd(nc, in_maps, list(range(N_CORES)),
                               trace=_trace, **(_trace_kwargs or {}))
    res = out.results
    y = np.empty((TOK, D), dtype=np.float32)
    for c in range(N_CORES):
        ytc = np.asarray(res[c]["yt"]).astype(np.float32)   # [3072, 394]
        y[c * TOK_PER:(c + 1) * TOK_PER] = (
            ytc.reshape(NT, KD, 128, TN).transpose(0, 3, 1, 2).reshape(TOK_PER, D))
    y = y.reshape(B, S, D)
    if _trace:
        return y, out
    return y
